# revision 1
# baseline (speedup 1.0000x reference)
"""DeepAR LSTM kernel for 8 Trainium2 NeuronCores.

Data-parallel over batch (256 -> 8 cores x 32). Latency-oriented design:
the recurrence is a serial chain PE -> ACT -> DVE -> ACT -> DVE per step,
so everything else is moved off that chain.

  * fp16 matmul operands everywhere (1 PE cycle/row vs 4 for fp32);
    PSUM accumulation stays fp32.
  * Gate pre-activations accumulate in a persistent 16-step PSUM ring
    [128, 16*4*32]. The x-side matmuls for step t depend only on x (DMA'd
    up front), so the PE executes them early; only the 4 h-side matmuls
    (start=False accumulate) sit on the critical path.
  * One sigmoid covers all 4 gates: the g-gate rows of W/bias are
    pre-scaled by 2 on the host, and tanh(a) = 2*sigmoid(2a) - 1 is fixed
    up in DVE with a single tensor_scalar op.
  * Bias is folded into the x-side matmul via a constant-1 row of x.
  * mu/logsigma heads run as one PE matmul per step (stationary = h_t),
    accumulating into a PSUM ring that is DMA'd straight to DRAM every
    4 steps; head biases are added on the host.
"""

import os
import sys
from contextlib import ExitStack

import numpy as np

sys.path.insert(0, "/opt/trn_rl_repo")

import concourse.bass as bass
import concourse.tile as tile
from concourse import bacc, mybir
from concourse.bass_utils import run_bass_kernel_spmd

L, B, IN, K, OBS = 1024, 256, 64, 128, 32
NCORES = 8
BL = B // NCORES   # 32 batch rows per core
SLOTS = 16         # gate-psum ring depth (steps)
HSLOTS = 16        # heads-psum ring depth (steps)

_LSTEPS = int(os.environ.get("KERNEL_LSTEPS", L))  # smoke-test override

F32 = mybir.dt.float32
F16 = mybir.dt.float16
AF = mybir.ActivationFunctionType
OP = mybir.AluOpType

_cache = {}
RUN_KW = {}         # test harness may inject trace=True/tmpdir
LAST_RESULT = None  # BassKernelResults of the most recent run


def build_nc(nsteps: int) -> bass.Bass:
    nc = bacc.Bacc(
        "TRN2", target_bir_lowering=False, debug=False, num_devices=NCORES
    )
    xt = nc.dram_tensor("xt", [IN + 1, nsteps * BL], F16, kind="ExternalInput")
    whh = nc.dram_tensor("whh_t", [K, 4 * K], F16, kind="ExternalInput")
    wih = nc.dram_tensor("wih_t", [IN + 1, 4 * K], F16, kind="ExternalInput")
    whd = nc.dram_tensor("wheads", [K, 2 * OBS], F16, kind="ExternalInput")
    heads = nc.dram_tensor(
        "heads", [BL, nsteps * 2 * OBS], F32, kind="ExternalOutput"
    )

    with ExitStack() as ctx:
        tc = ctx.enter_context(tile.TileContext(nc))
        singles = ctx.enter_context(tc.tile_pool(name="singles", bufs=1))
        gpsp = ctx.enter_context(tc.tile_pool(name="gps", bufs=1, space="PSUM"))
        hpsp = ctx.enter_context(tc.tile_pool(name="hps", bufs=1, space="PSUM"))
        dpsp = ctx.enter_context(tc.tile_pool(name="dps", bufs=1, space="PSUM"))

        whh_sb = singles.tile([K, 4 * K], F16)
        nc.sync.dma_start(whh_sb[:], whh[:])
        wih_sb = singles.tile([IN + 1, 4 * K], F16)
        nc.sync.dma_start(wih_sb[:], wih[:])
        whd_sb = singles.tile([K, 2 * OBS], F16)
        nc.sync.dma_start(whd_sb[:], whd[:])
        xt_sb = singles.tile([IN + 1, nsteps * BL], F16)
        nc.sync.dma_start(xt_sb[:], xt[:])

        sgt = [singles.tile([K, 4 * BL], F16, name=f"sg{i}") for i in range(2)]
        ct = [singles.tile([K, BL], F16, name=f"c{i}") for i in range(2)]
        tht = [singles.tile([K, BL], F16, name=f"th{i}") for i in range(2)]
        ht = [singles.tile([K, BL], F16, name=f"h{i}") for i in range(2)]
        g2 = singles.tile([K, BL], F16)
        ig = singles.tile([K, BL], F16)
        fc = singles.tile([K, BL], F16)
        stgt = [
            singles.tile([BL, 8 * 2 * OBS], F32, name=f"stg{i}")
            for i in range(2)
        ]

        gates_ps = gpsp.tile([K, SLOTS * 4 * BL], F32)    # 4 PSUM banks
        heads_ps = hpsp.tile([BL, HSLOTS * 2 * OBS], F32)  # 1 PSUM bank

        # A matmul can carry only ONE sync wait; make PE observe each DMA
        # semaphore via a throwaway 1x1 matmul so real matmuls never need
        # a DMA wait on top of a data-dependency wait.
        dummy_ps = dpsp.tile([1, 1], F32)
        absorb_state = {"first": True}

        def pe_absorb(tile_ap):
            nc.tensor.matmul(
                dummy_ps[:], tile_ap[0:1, 0:1], tile_ap[0:1, 0:1],
                start=absorb_state["first"], stop=False,
                skip_group_check=True,
            )
            absorb_state["first"] = False

        pe_absorb(whh_sb)
        pe_absorb(wih_sb)
        pe_absorb(whd_sb)
        pe_absorb(xt_sb)

        last_flush = -1
        for t in range(nsteps):
            base = (t % SLOTS) * 4 * BL
            xs = xt_sb[:, t * BL : (t + 1) * BL]
            # x-side matmuls: no h dependency -> execute early, off the
            # critical path. First one carries the WAR wait vs the sigmoid
            # read of this slot 16 steps ago.
            #
            # start=True marks the whole 2KB PSUM bank (4 slots) as
            # pending-zero; writes to marked bytes overwrite and clear the
            # mark, writes to cleared bytes accumulate. So assert start
            # only on the first matmul touching each bank (t%4==0, g==0);
            # every other x-matmul overwrites via its still-pending mark
            # and every h-matmul accumulates.
            for g in range(4):
                nc.tensor.matmul(
                    gates_ps[:, base + g * BL : base + (g + 1) * BL],
                    wih_sb[:, g * K : (g + 1) * K], xs,
                    start=(t % 4 == 0 and g == 0), stop=False,
                    skip_group_check=True,
                )
            if t > 0:
                hprev = ht[(t - 1) % 2]
                for g in range(4):
                    nc.tensor.matmul(
                        gates_ps[:, base + g * BL : base + (g + 1) * BL],
                        whh_sb[:, g * K : (g + 1) * K], hprev[:],
                        start=False, stop=(g == 3), skip_group_check=True,
                    )
                # heads matmul for h_{t-1}: stationary = h (free 32 ->
                # out partitions 32), moving = [W_mu.T | W_sig.T].
                j = t - 1
                hbase = (j % HSLOTS) * 2 * OBS
                nc.tensor.matmul(
                    heads_ps[:, hbase : hbase + 2 * OBS],
                    hprev[:], whd_sb[:],
                    start=True, stop=True, skip_group_check=True,
                )
            sg = sgt[t % 2]
            nc.scalar.activation(
                sg[:], gates_ps[:, base : base + 4 * BL], AF.Sigmoid
            )
            cnew = ct[t % 2]
            # gate layout in sg: i | f | o | g'  with  g = 2*sigmoid-1
            nc.vector.tensor_scalar(
                g2[:], sg[:, 3 * BL : 4 * BL], 2.0, 1.0, OP.mult, OP.subtract
            )
            if t == 0:
                nc.vector.tensor_mul(cnew[:], sg[:, 0:BL], g2[:])
            else:
                cprev = ct[(t - 1) % 2]
                nc.vector.tensor_mul(ig[:], sg[:, 0:BL], g2[:])
                nc.vector.tensor_mul(fc[:], sg[:, BL : 2 * BL], cprev[:])
                nc.vector.tensor_add(cnew[:], ig[:], fc[:])
            th = tht[t % 2]
            nc.scalar.activation(th[:], cnew[:], AF.Tanh)
            nc.vector.tensor_mul(ht[t % 2][:], sg[:, 2 * BL : 3 * BL], th[:])
            # flush completed 4-step heads groups: PSUM -> SBUF staging on
            # the otherwise-idle GPSIMD engine, then DMA to DRAM.
            j = t - 1
            if t > 0 and j % 8 == 7:
                s0 = ((j - 7) % HSLOTS) * 2 * OBS
                stg = stgt[(j // 8) % 2]
                if (j // 8) % 2 == 0:
                    nc.vector.tensor_copy(
                        stg[:], heads_ps[:, s0 : s0 + 8 * 2 * OBS]
                    )
                else:
                    nc.scalar.copy(
                        stg[:], heads_ps[:, s0 : s0 + 8 * 2 * OBS]
                    )
                nc.sync.dma_start(
                    heads[:, (j - 7) * 2 * OBS : (j + 1) * 2 * OBS], stg[:]
                )
                last_flush = j

        # final heads matmul + tail flush
        j = nsteps - 1
        hbase = (j % HSLOTS) * 2 * OBS
        nc.tensor.matmul(
            heads_ps[:, hbase : hbase + 2 * OBS],
            ht[j % 2][:], whd_sb[:],
            start=True, stop=True, skip_group_check=True,
        )
        f0 = last_flush + 1
        s0 = (f0 % HSLOTS) * 2 * OBS
        n = nsteps - f0
        stg = stgt[(f0 // 8) % 2]
        nc.vector.tensor_copy(
            stg[:, : n * 2 * OBS], heads_ps[:, s0 : s0 + n * 2 * OBS]
        )
        nc.sync.dma_start(
            heads[:, f0 * 2 * OBS : nsteps * 2 * OBS], stg[:, : n * 2 * OBS]
        )
    nc.compile()
    return nc


def _prep_weights(W_ih, W_hh, b_ih, b_hh, W_mu, W_sig):
    # torch gate order in rows: i(0:K) f(K:2K) g(2K:3K) o(3K:4K)
    # reorder rows to (i, f, o, g); scale the g block by 2 so one sigmoid
    # covers all gates (tanh(a) = 2*sigmoid(2a) - 1).
    perm = np.r_[0:K, K : 2 * K, 3 * K : 4 * K, 2 * K : 3 * K]
    whh_t = np.ascontiguousarray(W_hh[perm].T, np.float32)          # [K, 4K]
    bias = (b_ih + b_hh)[perm].astype(np.float32)
    wih_t = np.concatenate(
        [W_ih[perm].T, bias[None, :]], axis=0
    ).astype(np.float32)                                            # [IN+1, 4K]
    whh_t[:, 3 * K :] *= 2.0
    wih_t[:, 3 * K :] *= 2.0
    wheads = np.concatenate([W_mu.T, W_sig.T], axis=1).astype(np.float32)
    return (
        whh_t.astype(np.float16),
        wih_t.astype(np.float16),
        wheads.astype(np.float16),
    )


def kernel(external_input_seq, W_ih, W_hh, b_ih, b_hh, W_mu, b_mu, W_sig, b_sig):
    nsteps = _LSTEPS
    x = np.asarray(external_input_seq, np.float32)[:nsteps]
    W_ih = np.asarray(W_ih, np.float32)
    W_hh = np.asarray(W_hh, np.float32)
    b_ih = np.asarray(b_ih, np.float32)
    b_hh = np.asarray(b_hh, np.float32)
    W_mu = np.asarray(W_mu, np.float32)
    b_mu = np.asarray(b_mu, np.float32)
    W_sig = np.asarray(W_sig, np.float32)
    b_sig = np.asarray(b_sig, np.float32)

    whh_t, wih_t, wheads = _prep_weights(W_ih, W_hh, b_ih, b_hh, W_mu, W_sig)

    if nsteps not in _cache:
        _cache[nsteps] = build_nc(nsteps)
    nc = _cache[nsteps]

    in_maps = []
    for c in range(NCORES):
        xc = x[:, c * BL : (c + 1) * BL, :]              # [nsteps, BL, IN]
        xt = np.empty((IN + 1, nsteps * BL), np.float16)
        xt[:IN] = xc.transpose(2, 0, 1).reshape(IN, nsteps * BL)
        xt[IN] = 1.0
        in_maps.append(
            {"xt": xt, "whh_t": whh_t, "wih_t": wih_t, "wheads": wheads}
        )

    res = run_bass_kernel_spmd(
        nc, in_maps, core_ids=list(range(NCORES)), **RUN_KW
    )
    global LAST_RESULT
    LAST_RESULT = res

    mu = np.empty((nsteps, B, OBS), np.float32)
    sig = np.empty((nsteps, B, OBS), np.float32)
    for c in range(NCORES):
        h = res.results[c]["heads"].reshape(BL, nsteps, 2 * OBS)
        mu[:, c * BL : (c + 1) * BL, :] = h[:, :, :OBS].transpose(1, 0, 2)
        sig[:, c * BL : (c + 1) * BL, :] = h[:, :, OBS:].transpose(1, 0, 2)
    mu += b_mu
    sig += b_sig
    return mu, sig



# revision 7
# speedup vs baseline: 1.0693x; 1.0693x over previous
"""DeepAR LSTM kernel for 8 Trainium2 NeuronCores.

Data-parallel over batch (256 -> 8 cores x 32). The recurrence is a
latency-bound serial chain; per step the critical cycle is
PE(4 matmuls) -> ACT -> DVE -> ACT -> DVE -> PE. Instruction fixed
costs dominate (ACT ~300ns, DVE ~175ns per op), so the design minimizes
chain instruction count:

  * All four gates reparametrized as tanh: i = (tanh(a_i/2)+1)/2 etc.
    (host prescales the i/f/o rows of W/bias by 0.5), so ONE ACTIVATE
    covers all gates and the cell update needs only scalar_tensor_tensor
    ops. With d := 2c as the carry and H := 2h (factor folded into
    W_hh/W_heads on the host):
       [q|p] = ([F|A] + 1) * [d_prev|G]     one fused stt (FD=64)
       d     = 0.5*q + p                    one stt
       th    = tanh(0.5*d)                  ACT, free affine, dst PSUM
       H     = (O+1) * th                   one stt -> h_all
  * Gate pre-activations accumulate in a 16-step PSUM ring; x-side
    matmuls and bias (via constant-1 row of x) are issued early, only
    the 4 h-side matmuls sit on the critical path.
  * fp16 operands everywhere; PSUM accumulation fp32.
  * Heads (mu/logsigma) batched: every 16 steps one PE matmul over the
    h_all history chunk -> PSUM -> DVE copy -> DMA. Nothing per-step on
    the chain engines; head biases added on the host.
"""

import os
import sys
from contextlib import ExitStack

import numpy as np

sys.path.insert(0, "/opt/trn_rl_repo")

import concourse.bass as bass
import concourse.tile as tile
from concourse import bacc, mybir
from concourse.ap import AP
from concourse.bass_utils import run_bass_kernel_spmd

L, B, IN, K, OBS = 1024, 256, 64, 128, 32
NCORES = 8
BL = B // NCORES   # 32 batch rows per core
SLOTS = 16         # gates psum ring depth (steps)
R = 4              # SBUF cell ring depth (steps)
SW = 224           # ring slot width: T(128) | d(32) | q(32) | p(32)
HCH = 16           # heads chunk (steps per heads matmul)

_LSTEPS = int(os.environ.get("KERNEL_LSTEPS", L))  # smoke-test override
TH_SBUF = os.environ.get("KERNEL_TH_SBUF", "0") == "1"

F32 = mybir.dt.float32
F16 = mybir.dt.float16
AF = mybir.ActivationFunctionType
OP = mybir.AluOpType

_cache = {}
RUN_KW = {}         # test harness may inject trace=True/tmpdir
LAST_RESULT = None  # BassKernelResults of the most recent run


def build_nc(nsteps: int) -> bass.Bass:
    assert nsteps % HCH == 0
    nchunks = nsteps // HCH
    nc = bacc.Bacc(
        "TRN2", target_bir_lowering=False, debug=False, num_devices=NCORES
    )
    xt = nc.dram_tensor("xt", [IN + 1, nsteps * BL], F16, kind="ExternalInput")
    whh = nc.dram_tensor("whh_t", [K, 4 * K], F16, kind="ExternalInput")
    wih = nc.dram_tensor("wih_t", [IN + 1, 4 * K], F16, kind="ExternalInput")
    whd = nc.dram_tensor("wheads", [K, 2 * OBS], F16, kind="ExternalInput")
    heads = nc.dram_tensor(
        "heads", [2 * OBS, nsteps * BL], F16, kind="ExternalOutput"
    )

    with ExitStack() as ctx:
        tc = ctx.enter_context(tile.TileContext(nc))
        singles = ctx.enter_context(tc.tile_pool(name="singles", bufs=1))
        gpsp = ctx.enter_context(tc.tile_pool(name="gps", bufs=1, space="PSUM"))
        hpsp = ctx.enter_context(tc.tile_pool(name="hps", bufs=1, space="PSUM"))
        tpsp = ctx.enter_context(tc.tile_pool(name="tps", bufs=1, space="PSUM"))
        dpsp = ctx.enter_context(tc.tile_pool(name="dps", bufs=1, space="PSUM"))

        whh_sb = singles.tile([K, 4 * K], F16)
        nc.sync.dma_start(whh_sb[:], whh[:])
        wih_sb = singles.tile([IN + 1, 4 * K], F16)
        nc.sync.dma_start(wih_sb[:], wih[:])
        whd_sb = singles.tile([K, 2 * OBS], F16)
        nc.sync.dma_start(whd_sb[:], whd[:])
        xt_sb = singles.tile([IN + 1, nsteps * BL], F16)
        nc.sync.dma_start(xt_sb[:], xt[:])

        # cell ring: per slot r at r*SW: [T(128) | d(32) | q(32) | p(32)]
        # T gate order: F | A | O | G  (f, i, o, g)
        ring = singles.tile([K, R * SW], F16)
        h_all = singles.tile([K, nsteps * BL], F16)
        stgt = [
            singles.tile([2 * OBS, HCH * BL], F16, name=f"stg{i}")
            for i in range(2)
        ]

        gates_ps = gpsp.tile([K, SLOTS * 4 * BL], F32)      # 4 PSUM banks
        heads_ps = hpsp.tile([2 * OBS, 2 * HCH * BL], F32)  # 2 PSUM banks
        th_ps = tpsp.tile([K, 2 * BL], F32)                 # 1 PSUM bank
        th_sb = singles.tile([K, 2 * BL], F16)

        # A matmul can carry only ONE sync wait; make PE observe each DMA
        # semaphore via a throwaway 1x1 matmul so real matmuls never need
        # a DMA wait on top of a data-dependency wait.
        dummy_ps = dpsp.tile([1, 1], F32)
        absorb_state = {"first": True}

        def pe_absorb(tile_ap):
            nc.tensor.matmul(
                dummy_ps[:], tile_ap[0:1, 0:1], tile_ap[0:1, 0:1],
                start=absorb_state["first"], stop=False,
                skip_group_check=True,
            )
            absorb_state["first"] = False

        pe_absorb(whh_sb)
        pe_absorb(wih_sb)
        pe_absorb(whd_sb)
        pe_absorb(xt_sb)

        # zero d-slot of ring slot R-1 (d_{-1} = 0 for step 0)
        nc.vector.memset(ring[:, (R - 1) * SW + 128 : (R - 1) * SW + 160], 0)

        rt = ring.tensor
        rw = R * SW  # ring free width (elements per partition)

        def rap(off, dims):
            return AP(rt, off, [[rw, K]] + dims)

        for t in range(nsteps):
            r = (t % R) * SW
            r1 = ((t - 1) % R) * SW
            s = (t % SLOTS) * 4 * BL
            xs = xt_sb[:, t * BL : (t + 1) * BL]
            # x-side matmuls + bias: no h dependency -> execute early.
            # start=True only on the first matmul touching each 4-slot
            # PSUM bank (marks bank pending-zero; x overwrites, h accums).
            for g in range(4):
                nc.tensor.matmul(
                    gates_ps[:, s + g * BL : s + (g + 1) * BL],
                    wih_sb[:, g * K : (g + 1) * K], xs,
                    start=(t % 4 == 0 and g == 0),
                    stop=(t == 0 and g == 3),
                    skip_group_check=True,
                )
            if t > 0:
                hprev = h_all[:, (t - 1) * BL : t * BL]
                for g in range(4):
                    nc.tensor.matmul(
                        gates_ps[:, s + g * BL : s + (g + 1) * BL],
                        whh_sb[:, g * K : (g + 1) * K], hprev,
                        start=False, stop=(g == 3), skip_group_check=True,
                    )
            # heads chunk c = steps [c*HCH, (c+1)*HCH), 2 steps of slack
            if t >= HCH + 2 and (t - 2) % HCH == 0:
                c = (t - 2) // HCH - 1
                nc.tensor.matmul(
                    heads_ps[:, (c % 2) * HCH * BL : (c % 2 + 1) * HCH * BL],
                    whd_sb[:], h_all[:, c * HCH * BL : (c + 1) * HCH * BL],
                    start=True, stop=True, skip_group_check=True,
                )
            # chain: T = tanh(gates)
            nc.scalar.activation(
                ring[:, r : r + 128], gates_ps[:, s : s + 4 * BL], AF.Tanh
            )
            # [q|p] = ([F|A] + 1) * [d_prev|G]
            nc.vector.scalar_tensor_tensor(
                rap(r + 160, [[32, 2], [1, BL]]),
                rap(r, [[32, 2], [1, BL]]),
                1.0,
                rap(r1 + 128, [[(r + 96) - (r1 + 128), 2], [1, BL]]),
                OP.add, OP.mult,
            )
            # d = 0.5*q + p
            nc.vector.scalar_tensor_tensor(
                ring[:, r + 128 : r + 160],
                ring[:, r + 160 : r + 192],
                0.5,
                ring[:, r + 192 : r + 224],
                OP.mult, OP.add,
            )
            # th = tanh(0.5*d) -> PSUM (ScE->PSUM is the fast path)
            th = th_sb[:, (t % 2) * BL : (t % 2 + 1) * BL] if TH_SBUF else \
                th_ps[:, (t % 2) * BL : (t % 2 + 1) * BL]
            nc.scalar.activation(
                th, ring[:, r + 128 : r + 160], AF.Tanh, scale=0.5
            )
            # H = (O+1) * th -> h_all
            nc.vector.scalar_tensor_tensor(
                h_all[:, t * BL : (t + 1) * BL],
                ring[:, r + 64 : r + 96],
                1.0,
                th,
                OP.add, OP.mult,
            )
            # heads chunk evacuation, off the critical engines' busy slots
            if t >= HCH + 3 and (t - 3) % HCH == 0:
                c = (t - 3) // HCH - 1
                nc.vector.tensor_copy(
                    stgt[c % 2][:],
                    heads_ps[:, (c % 2) * HCH * BL : (c % 2 + 1) * HCH * BL],
                )
            if t >= HCH + 4 and (t - 4) % HCH == 0:
                c = (t - 4) // HCH - 1
                nc.sync.dma_start(
                    heads[:, c * HCH * BL : (c + 1) * HCH * BL], stgt[c % 2][:]
                )

        # tail: heads chunks whose DMA did not fire in-loop
        cdone = (nsteps - 5 - HCH) // HCH + 1 if nsteps >= HCH + 5 else 0
        for c in range(max(cdone, 0), nchunks):
            nc.tensor.matmul(
                heads_ps[:, (c % 2) * HCH * BL : (c % 2 + 1) * HCH * BL],
                whd_sb[:], h_all[:, c * HCH * BL : (c + 1) * HCH * BL],
                start=True, stop=True, skip_group_check=True,
            )
            nc.vector.tensor_copy(
                stgt[c % 2][:],
                heads_ps[:, (c % 2) * HCH * BL : (c % 2 + 1) * HCH * BL],
            )
            nc.sync.dma_start(
                heads[:, c * HCH * BL : (c + 1) * HCH * BL], stgt[c % 2][:]
            )
    nc.compile()
    return nc


def _prep_weights(W_ih, W_hh, b_ih, b_hh, W_mu, W_sig):
    # torch gate order in rows: i(0:K) f(K:2K) g(2K:3K) o(3K:4K)
    # reorder rows to (f, i, o, g); tanh-reparametrize i/f/o (prescale by
    # 0.5: sigma(x) = (tanh(x/2)+1)/2); W_hh and W_heads additionally
    # halved because the kernel's recurrent state is H = 2h.
    perm = np.r_[K : 2 * K, 0:K, 3 * K : 4 * K, 2 * K : 3 * K]
    gate_scale = np.concatenate(
        [np.full(3 * K, 0.5, np.float32), np.ones(K, np.float32)]
    )
    whh_t = np.ascontiguousarray(W_hh[perm].T, np.float32)          # [K, 4K]
    whh_t *= gate_scale[None, :] * 0.5
    bias = (b_ih + b_hh)[perm].astype(np.float32) * gate_scale
    wih_t = np.concatenate(
        [W_ih[perm].T * gate_scale[None, :], bias[None, :]], axis=0
    ).astype(np.float32)                                            # [IN+1, 4K]
    wheads = 0.5 * np.concatenate([W_mu.T, W_sig.T], axis=1).astype(np.float32)
    return (
        whh_t.astype(np.float16),
        wih_t.astype(np.float16),
        wheads.astype(np.float16),
    )


def kernel(external_input_seq, W_ih, W_hh, b_ih, b_hh, W_mu, b_mu, W_sig, b_sig):
    nsteps = _LSTEPS
    x = np.asarray(external_input_seq, np.float32)[:nsteps]
    W_ih = np.asarray(W_ih, np.float32)
    W_hh = np.asarray(W_hh, np.float32)
    b_ih = np.asarray(b_ih, np.float32)
    b_hh = np.asarray(b_hh, np.float32)
    W_mu = np.asarray(W_mu, np.float32)
    b_mu = np.asarray(b_mu, np.float32)
    W_sig = np.asarray(W_sig, np.float32)
    b_sig = np.asarray(b_sig, np.float32)

    whh_t, wih_t, wheads = _prep_weights(W_ih, W_hh, b_ih, b_hh, W_mu, W_sig)

    if nsteps not in _cache:
        _cache[nsteps] = build_nc(nsteps)
    nc = _cache[nsteps]

    in_maps = []
    for c in range(NCORES):
        xc = x[:, c * BL : (c + 1) * BL, :]              # [nsteps, BL, IN]
        xt = np.empty((IN + 1, nsteps * BL), np.float16)
        xt[:IN] = xc.transpose(2, 0, 1).reshape(IN, nsteps * BL)
        xt[IN] = 1.0
        in_maps.append(
            {"xt": xt, "whh_t": whh_t, "wih_t": wih_t, "wheads": wheads}
        )

    res = run_bass_kernel_spmd(
        nc, in_maps, core_ids=list(range(NCORES)), **RUN_KW
    )
    global LAST_RESULT
    LAST_RESULT = res

    mu = np.empty((nsteps, B, OBS), np.float32)
    sig = np.empty((nsteps, B, OBS), np.float32)
    for c in range(NCORES):
        h = res.results[c]["heads"].astype(np.float32)
        h = h.reshape(2 * OBS, nsteps, BL)               # [2OBS, t, b]
        mu[:, c * BL : (c + 1) * BL, :] = h[:OBS].transpose(1, 2, 0)
        sig[:, c * BL : (c + 1) * BL, :] = h[OBS:].transpose(1, 2, 0)
    mu += b_mu
    sig += b_sig
    return mu, sig


# revision 10
# speedup vs baseline: 1.0700x; 1.0007x over previous
"""DeepAR LSTM kernel for 8 Trainium2 NeuronCores.

Data-parallel over batch (256 -> 8 cores x 32). The recurrence is a
latency-bound serial chain; per step the critical cycle is
PE(4 matmuls) -> ACT -> DVE -> ACT -> DVE -> PE. Instruction fixed
costs dominate (ACT ~300ns, DVE ~175ns per op), so the design minimizes
chain instruction count:

  * All four gates reparametrized as tanh: i = (tanh(a_i/2)+1)/2 etc.
    (host prescales the i/f/o rows of W/bias by 0.5), so ONE ACTIVATE
    covers all gates and the cell update needs only scalar_tensor_tensor
    ops. With d := 2c as the carry and H := 2h (factor folded into
    W_hh/W_heads on the host):
       [q|p] = ([F|A] + 1) * [d_prev|G]     one fused stt (FD=64)
       d     = 0.5*q + p                    one stt
       th    = tanh(0.5*d)                  ACT, free affine, dst PSUM
       H     = (O+1) * th                   one stt -> h_all
  * Gate pre-activations accumulate in a 16-step PSUM ring; x-side
    matmuls and bias (via constant-1 row of x) are issued early, only
    the 4 h-side matmuls sit on the critical path.
  * fp16 operands everywhere; PSUM accumulation fp32.
  * Heads (mu/logsigma) batched: every 16 steps one PE matmul over the
    h_all history chunk -> PSUM -> DVE copy -> DMA. Nothing per-step on
    the chain engines; head biases added on the host.
"""

import os
import sys
from contextlib import ExitStack

import numpy as np

sys.path.insert(0, "/opt/trn_rl_repo")

import concourse.bass as bass
import concourse.tile as tile
from concourse import bacc, mybir
from concourse.ap import AP
from concourse.bass_utils import run_bass_kernel_spmd

L, B, IN, K, OBS = 1024, 256, 64, 128, 32
NCORES = 8
BL = B // NCORES   # 32 batch rows per core
SLOTS = 4          # gates psum ring depth; one full PSUM bank per step
SSTRIDE = 512      # f32 elements per gates slot (= one 2KB bank)
R = 4              # SBUF cell ring depth (steps)
SW = 224           # ring slot width: T(128) | d(32) | q(32) | p(32)
HCH = 16           # heads chunk (steps per heads matmul)

_LSTEPS = int(os.environ.get("KERNEL_LSTEPS", L))  # smoke-test override
TH_SBUF = os.environ.get("KERNEL_TH_SBUF", "0") == "1"

F32 = mybir.dt.float32
F16 = mybir.dt.float16
AF = mybir.ActivationFunctionType
OP = mybir.AluOpType

_cache = {}
RUN_KW = {}         # test harness may inject trace=True/tmpdir
LAST_RESULT = None  # BassKernelResults of the most recent run


def build_nc(nsteps: int) -> bass.Bass:
    assert nsteps % HCH == 0
    nchunks = nsteps // HCH
    nc = bacc.Bacc(
        "TRN2", target_bir_lowering=False, debug=False, num_devices=NCORES
    )
    xt = nc.dram_tensor("xt", [IN + 1, nsteps * BL], F16, kind="ExternalInput")
    whh = nc.dram_tensor("whh_t", [K, 4 * K], F16, kind="ExternalInput")
    wih = nc.dram_tensor("wih_t", [IN + 1, 4 * K], F16, kind="ExternalInput")
    whd = nc.dram_tensor("wheads", [K, 2 * OBS], F16, kind="ExternalInput")
    heads = nc.dram_tensor(
        "heads", [2 * OBS, nsteps * BL], F16, kind="ExternalOutput"
    )

    with ExitStack() as ctx:
        tc = ctx.enter_context(tile.TileContext(nc))
        singles = ctx.enter_context(tc.tile_pool(name="singles", bufs=1))
        gpsp = ctx.enter_context(tc.tile_pool(name="gps", bufs=1, space="PSUM"))
        hpsp = ctx.enter_context(tc.tile_pool(name="hps", bufs=1, space="PSUM"))
        tpsp = ctx.enter_context(tc.tile_pool(name="tps", bufs=1, space="PSUM"))
        dpsp = ctx.enter_context(tc.tile_pool(name="dps", bufs=1, space="PSUM"))

        whh_sb = singles.tile([K, 4 * K], F16)
        nc.sync.dma_start(whh_sb[:], whh[:])
        wih_sb = singles.tile([IN + 1, 4 * K], F16)
        nc.sync.dma_start(wih_sb[:], wih[:])
        whd_sb = singles.tile([K, 2 * OBS], F16)
        nc.sync.dma_start(whd_sb[:], whd[:])
        xt_sb = singles.tile([IN + 1, nsteps * BL], F16)
        nc.sync.dma_start(xt_sb[:], xt[:])

        # cell ring: per slot r at r*SW: [T(128) | d(32) | q(32) | p(32)]
        # T gate order: F | A | O | G  (f, i, o, g)
        ring = singles.tile([K, R * SW], F16)
        h_all = singles.tile([K, nsteps * BL], F16)
        stgt = [
            singles.tile([2 * OBS, HCH * BL], F16, name=f"stg{i}")
            for i in range(2)
        ]

        gates_ps = gpsp.tile([K, SLOTS * SSTRIDE], F32)     # 4 PSUM banks
        heads_ps = hpsp.tile([2 * OBS, 2 * HCH * BL], F32)  # 2 PSUM banks
        th_ps = tpsp.tile([K, 2 * BL], F32)                 # 1 PSUM bank
        th_sb = singles.tile([K, 2 * BL], F16)

        # A matmul can carry only ONE sync wait; make PE observe each DMA
        # semaphore via a throwaway 1x1 matmul so real matmuls never need
        # a DMA wait on top of a data-dependency wait.
        dummy_ps = dpsp.tile([1, 1], F32)
        absorb_state = {"first": True}

        def pe_absorb(tile_ap):
            nc.tensor.matmul(
                dummy_ps[:], tile_ap[0:1, 0:1], tile_ap[0:1, 0:1],
                start=absorb_state["first"], stop=False,
                skip_group_check=True,
            )
            absorb_state["first"] = False

        pe_absorb(whh_sb)
        pe_absorb(wih_sb)
        pe_absorb(whd_sb)
        pe_absorb(xt_sb)

        # zero d-slot of ring slot R-1 (d_{-1} = 0 for step 0)
        nc.vector.memset(ring[:, (R - 1) * SW + 128 : (R - 1) * SW + 160], 0)

        rt = ring.tensor
        rw = R * SW  # ring free width (elements per partition)

        def rap(off, dims):
            return AP(rt, off, [[rw, K]] + dims)

        for t in range(nsteps):
            r = (t % R) * SW
            r1 = ((t - 1) % R) * SW
            s = (t % SLOTS) * SSTRIDE
            xs = xt_sb[:, t * BL : (t + 1) * BL]
            # x-side matmuls + bias: no h dependency -> execute early.
            # Each step owns a full PSUM bank, so concurrent PE writes and
            # ACT reads of neighboring steps never collide on a bank.
            # start=True marks the bank pending-zero; x overwrites, h accums.
            for g in range(4):
                nc.tensor.matmul(
                    gates_ps[:, s + g * BL : s + (g + 1) * BL],
                    wih_sb[:, g * K : (g + 1) * K], xs,
                    start=(g == 0),
                    stop=(t == 0 and g == 3),
                    skip_group_check=True,
                )
            if t > 0:
                hprev = h_all[:, (t - 1) * BL : t * BL]
                for g in range(4):
                    nc.tensor.matmul(
                        gates_ps[:, s + g * BL : s + (g + 1) * BL],
                        whh_sb[:, g * K : (g + 1) * K], hprev,
                        start=False, stop=(g == 3), skip_group_check=True,
                    )
            # heads chunk c = steps [c*HCH, (c+1)*HCH), 2 steps of slack
            if t >= HCH + 2 and (t - 2) % HCH == 0:
                c = (t - 2) // HCH - 1
                nc.tensor.matmul(
                    heads_ps[:, (c % 2) * HCH * BL : (c % 2 + 1) * HCH * BL],
                    whd_sb[:], h_all[:, c * HCH * BL : (c + 1) * HCH * BL],
                    start=True, stop=True, skip_group_check=True,
                )
            # chain: T = tanh(gates)
            nc.scalar.activation(
                ring[:, r : r + 128], gates_ps[:, s : s + 4 * BL], AF.Tanh
            )
            # [q|p] = ([F|A] + 1) * [d_prev|G]
            nc.vector.scalar_tensor_tensor(
                rap(r + 160, [[32, 2], [1, BL]]),
                rap(r, [[32, 2], [1, BL]]),
                1.0,
                rap(r1 + 128, [[(r + 96) - (r1 + 128), 2], [1, BL]]),
                OP.add, OP.mult,
            )
            # d = 0.5*q + p
            nc.vector.scalar_tensor_tensor(
                ring[:, r + 128 : r + 160],
                ring[:, r + 160 : r + 192],
                0.5,
                ring[:, r + 192 : r + 224],
                OP.mult, OP.add,
            )
            # th = tanh(0.5*d) -> PSUM (ScE->PSUM is the fast path)
            th = th_sb[:, (t % 2) * BL : (t % 2 + 1) * BL] if TH_SBUF else \
                th_ps[:, (t % 2) * BL : (t % 2 + 1) * BL]
            nc.scalar.activation(
                th, ring[:, r + 128 : r + 160], AF.Tanh, scale=0.5
            )
            # H = (O+1) * th -> h_all
            nc.vector.scalar_tensor_tensor(
                h_all[:, t * BL : (t + 1) * BL],
                ring[:, r + 64 : r + 96],
                1.0,
                th,
                OP.add, OP.mult,
            )
            # heads chunk evacuation, off the critical engines' busy slots
            if t >= HCH + 3 and (t - 3) % HCH == 0:
                c = (t - 3) // HCH - 1
                nc.vector.tensor_copy(
                    stgt[c % 2][:],
                    heads_ps[:, (c % 2) * HCH * BL : (c % 2 + 1) * HCH * BL],
                )
            if t >= HCH + 4 and (t - 4) % HCH == 0:
                c = (t - 4) // HCH - 1
                nc.sync.dma_start(
                    heads[:, c * HCH * BL : (c + 1) * HCH * BL], stgt[c % 2][:]
                )

        # tail: heads chunks whose DMA did not fire in-loop
        cdone = (nsteps - 5 - HCH) // HCH + 1 if nsteps >= HCH + 5 else 0
        for c in range(max(cdone, 0), nchunks):
            nc.tensor.matmul(
                heads_ps[:, (c % 2) * HCH * BL : (c % 2 + 1) * HCH * BL],
                whd_sb[:], h_all[:, c * HCH * BL : (c + 1) * HCH * BL],
                start=True, stop=True, skip_group_check=True,
            )
            nc.vector.tensor_copy(
                stgt[c % 2][:],
                heads_ps[:, (c % 2) * HCH * BL : (c % 2 + 1) * HCH * BL],
            )
            nc.sync.dma_start(
                heads[:, c * HCH * BL : (c + 1) * HCH * BL], stgt[c % 2][:]
            )
    nc.compile()
    return nc


def _prep_weights(W_ih, W_hh, b_ih, b_hh, W_mu, W_sig):
    # torch gate order in rows: i(0:K) f(K:2K) g(2K:3K) o(3K:4K)
    # reorder rows to (f, i, o, g); tanh-reparametrize i/f/o (prescale by
    # 0.5: sigma(x) = (tanh(x/2)+1)/2); W_hh and W_heads additionally
    # halved because the kernel's recurrent state is H = 2h.
    perm = np.r_[K : 2 * K, 0:K, 3 * K : 4 * K, 2 * K : 3 * K]
    gate_scale = np.concatenate(
        [np.full(3 * K, 0.5, np.float32), np.ones(K, np.float32)]
    )
    whh_t = np.ascontiguousarray(W_hh[perm].T, np.float32)          # [K, 4K]
    whh_t *= gate_scale[None, :] * 0.5
    bias = (b_ih + b_hh)[perm].astype(np.float32) * gate_scale
    wih_t = np.concatenate(
        [W_ih[perm].T * gate_scale[None, :], bias[None, :]], axis=0
    ).astype(np.float32)                                            # [IN+1, 4K]
    wheads = 0.5 * np.concatenate([W_mu.T, W_sig.T], axis=1).astype(np.float32)
    return (
        whh_t.astype(np.float16),
        wih_t.astype(np.float16),
        wheads.astype(np.float16),
    )


def kernel(external_input_seq, W_ih, W_hh, b_ih, b_hh, W_mu, b_mu, W_sig, b_sig):
    nsteps = _LSTEPS
    x = np.asarray(external_input_seq, np.float32)[:nsteps]
    W_ih = np.asarray(W_ih, np.float32)
    W_hh = np.asarray(W_hh, np.float32)
    b_ih = np.asarray(b_ih, np.float32)
    b_hh = np.asarray(b_hh, np.float32)
    W_mu = np.asarray(W_mu, np.float32)
    b_mu = np.asarray(b_mu, np.float32)
    W_sig = np.asarray(W_sig, np.float32)
    b_sig = np.asarray(b_sig, np.float32)

    whh_t, wih_t, wheads = _prep_weights(W_ih, W_hh, b_ih, b_hh, W_mu, W_sig)

    if nsteps not in _cache:
        _cache[nsteps] = build_nc(nsteps)
    nc = _cache[nsteps]

    in_maps = []
    for c in range(NCORES):
        xc = x[:, c * BL : (c + 1) * BL, :]              # [nsteps, BL, IN]
        xt = np.empty((IN + 1, nsteps * BL), np.float16)
        xt[:IN] = xc.transpose(2, 0, 1).reshape(IN, nsteps * BL)
        xt[IN] = 1.0
        in_maps.append(
            {"xt": xt, "whh_t": whh_t, "wih_t": wih_t, "wheads": wheads}
        )

    res = run_bass_kernel_spmd(
        nc, in_maps, core_ids=list(range(NCORES)), **RUN_KW
    )
    global LAST_RESULT
    LAST_RESULT = res

    mu = np.empty((nsteps, B, OBS), np.float32)
    sig = np.empty((nsteps, B, OBS), np.float32)
    for c in range(NCORES):
        h = res.results[c]["heads"].astype(np.float32)
        h = h.reshape(2 * OBS, nsteps, BL)               # [2OBS, t, b]
        mu[:, c * BL : (c + 1) * BL, :] = h[:OBS].transpose(1, 2, 0)
        sig[:, c * BL : (c + 1) * BL, :] = h[OBS:].transpose(1, 2, 0)
    mu += b_mu
    sig += b_sig
    return mu, sig


# revision 14
# speedup vs baseline: 1.0849x; 1.0140x over previous
"""DeepAR LSTM kernel for 8 Trainium2 NeuronCores.

Data-parallel over batch (256 -> 8 cores x 32). The recurrence is a
latency-bound serial chain; per step the critical cycle is
PE(4 matmuls) -> ACT -> DVE -> ACT -> DVE -> PE. Instruction fixed
costs dominate (ACT ~300ns, DVE ~175ns per op), so the design minimizes
chain instruction count:

  * All four gates reparametrized as tanh: i = (tanh(a_i/2)+1)/2 etc.
    (host prescales the i/f/o rows of W/bias by 0.5), so ONE ACTIVATE
    covers all gates and the cell update needs only scalar_tensor_tensor
    ops. With d := 2c as the carry and H := 2h (factor folded into
    W_hh/W_heads on the host):
       [q|p] = ([F|A] + 1) * [d_prev|G]     one fused stt (FD=64)
       d     = 0.5*q + p                    one stt
       th    = tanh(0.5*d)                  ACT, free affine, dst PSUM
       H     = (O+1) * th                   one stt -> h_all
  * Gate pre-activations accumulate in a 16-step PSUM ring; x-side
    matmuls and bias (via constant-1 row of x) are issued early, only
    the 4 h-side matmuls sit on the critical path.
  * fp16 operands everywhere; PSUM accumulation fp32.
  * Heads (mu/logsigma) batched: every 16 steps one PE matmul over the
    h_all history chunk -> PSUM -> DVE copy -> DMA. Nothing per-step on
    the chain engines; head biases added on the host.
"""

import os
import sys
from contextlib import ExitStack

import numpy as np

sys.path.insert(0, "/opt/trn_rl_repo")

import concourse.bass as bass
import concourse.tile as tile
from concourse import bacc, mybir
from concourse.ap import AP
from concourse.bass_utils import run_bass_kernel_spmd

L, B, IN, K, OBS = 1024, 256, 64, 128, 32
NCORES = 8
BL = B // NCORES   # 32 batch rows per core
SLOTS = 4          # gates psum ring depth; one full PSUM bank per step
SSTRIDE = 512      # f32 elements per gates slot (= one 2KB bank)
R = 6              # SBUF cell ring depth (steps)
SW = 224           # ring slot width: T(128) | d(32) | q(32) | p(32)
HCH = 16           # heads chunk (steps per heads matmul)

_LSTEPS = int(os.environ.get("KERNEL_LSTEPS", L))  # smoke-test override
TH_SBUF = os.environ.get("KERNEL_TH_SBUF", "0") == "1"

F32 = mybir.dt.float32
F16 = mybir.dt.float16
AF = mybir.ActivationFunctionType
OP = mybir.AluOpType

_cache = {}
RUN_KW = {}         # test harness may inject trace=True/tmpdir
LAST_RESULT = None  # BassKernelResults of the most recent run


def build_nc(nsteps: int) -> bass.Bass:
    assert nsteps % HCH == 0
    nchunks = nsteps // HCH
    nc = bacc.Bacc(
        "TRN2", target_bir_lowering=False, debug=False, num_devices=NCORES
    )
    xt = nc.dram_tensor("xt", [IN + 1, nsteps * BL], F16, kind="ExternalInput")
    whh = nc.dram_tensor("whh_t", [K, 4 * K], F16, kind="ExternalInput")
    wih = nc.dram_tensor("wih_t", [IN + 1, 4 * K], F16, kind="ExternalInput")
    whd = nc.dram_tensor("wheads", [K, 2 * OBS], F16, kind="ExternalInput")
    heads = nc.dram_tensor(
        "heads", [2 * OBS, nsteps * BL], F16, kind="ExternalOutput"
    )

    with ExitStack() as ctx:
        tc = ctx.enter_context(tile.TileContext(nc))
        singles = ctx.enter_context(tc.tile_pool(name="singles", bufs=1))
        gpsp = ctx.enter_context(tc.tile_pool(name="gps", bufs=1, space="PSUM"))
        hpsp = ctx.enter_context(tc.tile_pool(name="hps", bufs=1, space="PSUM"))
        tpsp = ctx.enter_context(tc.tile_pool(name="tps", bufs=1, space="PSUM"))
        dpsp = ctx.enter_context(tc.tile_pool(name="dps", bufs=1, space="PSUM"))

        whh_sb = singles.tile([K, 4 * K], F16)
        nc.sync.dma_start(whh_sb[:], whh[:])
        wih_sb = singles.tile([IN + 1, 4 * K], F16)
        nc.sync.dma_start(wih_sb[:], wih[:])
        whd_sb = singles.tile([K, 2 * OBS], F16)
        nc.sync.dma_start(whd_sb[:], whd[:])
        xt_sb = singles.tile([IN + 1, nsteps * BL], F16)
        nc.sync.dma_start(xt_sb[:], xt[:])

        # cell ring: per slot r at r*SW: [T(128) | d(32) | q(32) | p(32)]
        # T gate order: F | A | O | G  (f, i, o, g)
        ring = singles.tile([K, R * SW], F16)
        h_all = singles.tile([K, nsteps * BL], F16)
        stgt = [
            singles.tile([2 * OBS, HCH * BL], F16, name=f"stg{i}")
            for i in range(2)
        ]

        gates_ps = gpsp.tile([K, SLOTS * SSTRIDE], F32)     # 4 PSUM banks
        heads_ps = hpsp.tile([2 * OBS, 2 * HCH * BL], F32)  # 2 PSUM banks
        th_ps = tpsp.tile([K, 2 * BL], F32)                 # 1 PSUM bank
        th_sb = singles.tile([K, 2 * BL], F16)

        # A matmul can carry only ONE sync wait; make PE observe each DMA
        # semaphore via a throwaway 1x1 matmul so real matmuls never need
        # a DMA wait on top of a data-dependency wait.
        dummy_ps = dpsp.tile([1, 1], F32)
        absorb_state = {"first": True}

        def pe_absorb(tile_ap):
            nc.tensor.matmul(
                dummy_ps[:], tile_ap[0:1, 0:1], tile_ap[0:1, 0:1],
                start=absorb_state["first"], stop=False,
                skip_group_check=True,
            )
            absorb_state["first"] = False

        pe_absorb(whh_sb)
        pe_absorb(wih_sb)
        pe_absorb(whd_sb)
        pe_absorb(xt_sb)

        # zero d-slot of ring slot R-1 (d_{-1} = 0 for step 0)
        nc.vector.memset(ring[:, (R - 1) * SW + 128 : (R - 1) * SW + 160], 0)

        rt = ring.tensor
        rw = R * SW  # ring free width (elements per partition)

        def rap(off, dims):
            return AP(rt, off, [[rw, K]] + dims)

        for t in range(nsteps):
            r = (t % R) * SW
            r1 = ((t - 1) % R) * SW
            s = (t % SLOTS) * SSTRIDE
            xs = xt_sb[:, t * BL : (t + 1) * BL]
            # x-side matmuls + bias: no h dependency -> execute early.
            # Each step owns a full PSUM bank, so concurrent PE writes and
            # ACT reads of neighboring steps never collide on a bank.
            # start=True marks the bank pending-zero; x overwrites, h accums.
            for g in range(4):
                nc.tensor.matmul(
                    gates_ps[:, s + g * BL : s + (g + 1) * BL],
                    wih_sb[:, g * K : (g + 1) * K], xs,
                    start=(g == 0),
                    stop=(t == 0 and g == 3),
                    skip_group_check=True,
                )
            if t > 0:
                hprev = h_all[:, (t - 1) * BL : t * BL]
                for g in range(4):
                    nc.tensor.matmul(
                        gates_ps[:, s + g * BL : s + (g + 1) * BL],
                        whh_sb[:, g * K : (g + 1) * K], hprev,
                        start=False, stop=(g == 3), skip_group_check=True,
                    )
            # heads chunk c = steps [c*HCH, (c+1)*HCH), 2 steps of slack
            if t >= HCH + 2 and (t - 2) % HCH == 0:
                c = (t - 2) // HCH - 1
                nc.tensor.matmul(
                    heads_ps[:, (c % 2) * HCH * BL : (c % 2 + 1) * HCH * BL],
                    whd_sb[:], h_all[:, c * HCH * BL : (c + 1) * HCH * BL],
                    start=True, stop=True, skip_group_check=True,
                )
            # chain: [F|A|G] = tanh(gates f,i,g) -- o-gate done off-chain
            nc.scalar.activation(
                ring[:, r : r + 96], gates_ps[:, s : s + 3 * BL], AF.Tanh
            )
            # off-chain: O = tanh(a_o/2); only needed by H, ~500ns later
            nc.scalar.activation(
                ring[:, r + 96 : r + 128],
                gates_ps[:, s + 3 * BL : s + 4 * BL], AF.Tanh,
            )
            # [q|p] = ([F|A] + 1) * [d_prev|G]
            nc.vector.scalar_tensor_tensor(
                rap(r + 160, [[32, 2], [1, BL]]),
                rap(r, [[32, 2], [1, BL]]),
                1.0,
                rap(r1 + 128, [[(r + 64) - (r1 + 128), 2], [1, BL]]),
                OP.add, OP.mult,
            )
            # d = 0.5*q + p
            nc.vector.scalar_tensor_tensor(
                ring[:, r + 128 : r + 160],
                ring[:, r + 160 : r + 192],
                0.5,
                ring[:, r + 192 : r + 224],
                OP.mult, OP.add,
            )
            # th = tanh(0.5*d) -> PSUM (ScE->PSUM is the fast path)
            th = th_sb[:, (t % 2) * BL : (t % 2 + 1) * BL] if TH_SBUF else \
                th_ps[:, (t % 2) * BL : (t % 2 + 1) * BL]
            nc.scalar.activation(
                th, ring[:, r + 128 : r + 160], AF.Tanh, scale=0.5
            )
            # H = (O+1) * th -> h_all
            nc.vector.scalar_tensor_tensor(
                h_all[:, t * BL : (t + 1) * BL],
                ring[:, r + 96 : r + 128],
                1.0,
                th,
                OP.add, OP.mult,
            )
            # heads chunk evacuation, off the critical engines' busy slots
            if t >= HCH + 3 and (t - 3) % HCH == 0:
                c = (t - 3) // HCH - 1
                nc.vector.tensor_copy(
                    stgt[c % 2][:],
                    heads_ps[:, (c % 2) * HCH * BL : (c % 2 + 1) * HCH * BL],
                )
            if t >= HCH + 4 and (t - 4) % HCH == 0:
                c = (t - 4) // HCH - 1
                nc.sync.dma_start(
                    heads[:, c * HCH * BL : (c + 1) * HCH * BL], stgt[c % 2][:]
                )

        # tail: heads chunks whose DMA did not fire in-loop
        cdone = (nsteps - 5 - HCH) // HCH + 1 if nsteps >= HCH + 5 else 0
        for c in range(max(cdone, 0), nchunks):
            nc.tensor.matmul(
                heads_ps[:, (c % 2) * HCH * BL : (c % 2 + 1) * HCH * BL],
                whd_sb[:], h_all[:, c * HCH * BL : (c + 1) * HCH * BL],
                start=True, stop=True, skip_group_check=True,
            )
            nc.vector.tensor_copy(
                stgt[c % 2][:],
                heads_ps[:, (c % 2) * HCH * BL : (c % 2 + 1) * HCH * BL],
            )
            nc.sync.dma_start(
                heads[:, c * HCH * BL : (c + 1) * HCH * BL], stgt[c % 2][:]
            )
    nc.compile()
    return nc


def _prep_weights(W_ih, W_hh, b_ih, b_hh, W_mu, W_sig):
    # torch gate order in rows: i(0:K) f(K:2K) g(2K:3K) o(3K:4K)
    # reorder rows to (f, i, g, o); tanh-reparametrize i/f/o (prescale by
    # 0.5: sigma(x) = (tanh(x/2)+1)/2); W_hh and W_heads additionally
    # halved because the kernel's recurrent state is H = 2h.
    perm = np.r_[K : 2 * K, 0:K, 2 * K : 3 * K, 3 * K : 4 * K]
    gate_scale = np.concatenate(
        [
            np.full(2 * K, 0.5, np.float32),
            np.ones(K, np.float32),
            np.full(K, 0.5, np.float32),
        ]
    )
    whh_t = np.ascontiguousarray(W_hh[perm].T, np.float32)          # [K, 4K]
    whh_t *= gate_scale[None, :] * 0.5
    bias = (b_ih + b_hh)[perm].astype(np.float32) * gate_scale
    wih_t = np.concatenate(
        [W_ih[perm].T * gate_scale[None, :], bias[None, :]], axis=0
    ).astype(np.float32)                                            # [IN+1, 4K]
    wheads = 0.5 * np.concatenate([W_mu.T, W_sig.T], axis=1).astype(np.float32)
    return (
        whh_t.astype(np.float16),
        wih_t.astype(np.float16),
        wheads.astype(np.float16),
    )


def kernel(external_input_seq, W_ih, W_hh, b_ih, b_hh, W_mu, b_mu, W_sig, b_sig):
    nsteps = _LSTEPS
    x = np.asarray(external_input_seq, np.float32)[:nsteps]
    W_ih = np.asarray(W_ih, np.float32)
    W_hh = np.asarray(W_hh, np.float32)
    b_ih = np.asarray(b_ih, np.float32)
    b_hh = np.asarray(b_hh, np.float32)
    W_mu = np.asarray(W_mu, np.float32)
    b_mu = np.asarray(b_mu, np.float32)
    W_sig = np.asarray(W_sig, np.float32)
    b_sig = np.asarray(b_sig, np.float32)

    whh_t, wih_t, wheads = _prep_weights(W_ih, W_hh, b_ih, b_hh, W_mu, W_sig)

    if nsteps not in _cache:
        _cache[nsteps] = build_nc(nsteps)
    nc = _cache[nsteps]

    in_maps = []
    for c in range(NCORES):
        xc = x[:, c * BL : (c + 1) * BL, :]              # [nsteps, BL, IN]
        xt = np.empty((IN + 1, nsteps * BL), np.float16)
        xt[:IN] = xc.transpose(2, 0, 1).reshape(IN, nsteps * BL)
        xt[IN] = 1.0
        in_maps.append(
            {"xt": xt, "whh_t": whh_t, "wih_t": wih_t, "wheads": wheads}
        )

    res = run_bass_kernel_spmd(
        nc, in_maps, core_ids=list(range(NCORES)), **RUN_KW
    )
    global LAST_RESULT
    LAST_RESULT = res

    mu = np.empty((nsteps, B, OBS), np.float32)
    sig = np.empty((nsteps, B, OBS), np.float32)
    for c in range(NCORES):
        h = res.results[c]["heads"].astype(np.float32)
        h = h.reshape(2 * OBS, nsteps, BL)               # [2OBS, t, b]
        mu[:, c * BL : (c + 1) * BL, :] = h[:OBS].transpose(1, 2, 0)
        sig[:, c * BL : (c + 1) * BL, :] = h[OBS:].transpose(1, 2, 0)
    mu += b_mu
    sig += b_sig
    return mu, sig
